# revision 8
# baseline (speedup 1.0000x reference)
"""BPLoss Trainium2 kernel: 8-core SPMD over the detection (N) axis.

v4 design. Per core (shard of R=12544 rows; partition p owns rows
p*98..p*98+97, so each group DMA reads one contiguous 28 KiB run per
partition):
  - 14 groups x [128, 7, 1024] f32 plain HWDGE DMAs (3.5 MiB each),
    alternating between the SP and ACT hardware DGE rings, stream
    class_scores at line rate (~330 GB/s measured)
  - masking per 128-row tile (HW-microbenched assignments):
      tiles 0-1 of each group: DVE fused stt
          masked = (iota != label) * cs -> scratch      (1.28 us)
      tiles 2-6: Pool in-place multiply by a bf16 {0,1} mask
          cs *= nv                                      (2.12 us)
        nv is built by ACT in 2 passes (Square(iota-label) -> Sign,
        1.13 us each) for most tiles; on even groups tile 2's nv comes
        from a DVE tensor_scalar (0.74 us) to keep ACT under budget.
        nv depends only on the label table, so mask builds run ahead of
        the data DMAs
  - DVE per-tile reduce_max for scratch tiles, one grouped reduce for
    the in-place tiles; all engines land at ~10.5-10.7 us per 11.2 us
    DMA group
  - epilogue: Ln on ScalarE, fused multiply-accumulate dot products for
    sum((z+r)*log_max) and sum(z*||xywh - gt_xywh[idx]||^2)
Host: gathers the tiny gt tables per row (labels, gt_xywh[idx]), shards,
pads core 7, sums the 8x[128,2] partials, combines -A + exp(-B).
"""
import numpy as np
import concourse.bass as bass
import concourse.tile as tile
from concourse import bacc, mybir
from concourse.bass_utils import run_bass_kernel_spmd

N, C, M = 100000, 1024, 128
NCORES = 8
T = 98              # 128-row tiles per core
R = T * 128         # 12544 rows per core
G = 7               # tiles per DMA group
NG = T // G         # 14 groups
DVE_APPLY = 2       # leading tiles of each group masked on DVE
CS_BUFS = 4
NV_BUFS = 10
MSK_BUFS = 4
BIG = 1024.0

f32 = mybir.dt.float32
bf16 = mybir.dt.bfloat16
OP = mybir.AluOpType
AF = mybir.ActivationFunctionType
AX = mybir.AxisListType

# packed f32 per-row tables: [label | -label | z | r | xywh | g | iota]
PF_LAB = 0
PF_NLAB = T
PF_Z = 2 * T
PF_R = 3 * T
PF_XYWH = 4 * T
PF_G = 8 * T
PF_IOTA = 12 * T
PF_COLS = 12 * T + C


def build_nc(reps=1):
    nc = bacc.Bacc("TRN2", target_bir_lowering=False, debug=False,
                   num_devices=NCORES)
    cs_d = nc.dram_tensor("cs", [128, T * C], f32, kind="ExternalInput").ap()
    pf_d = nc.dram_tensor("pf", [128, PF_COLS], f32, kind="ExternalInput").ap()
    out_d = nc.dram_tensor("out", [128, 2], f32, kind="ExternalOutput").ap()

    with tile.TileContext(nc) as tc:
        with (
            tc.tile_pool(name="const", bufs=1) as constp,
            tc.tile_pool(name="csp", bufs=CS_BUFS) as csp,
            tc.tile_pool(name="nvp", bufs=NV_BUFS) as nvp,
            tc.tile_pool(name="mskp", bufs=MSK_BUFS) as mskp,
        ):
            pf = constp.tile([128, PF_COLS], f32)
            nc.scalar.dma_start(out=pf[:], in_=pf_d[:])
            lab = pf[:, PF_LAB : PF_LAB + T]
            nlab = pf[:, PF_NLAB : PF_NLAB + T]
            z_sb = pf[:, PF_Z : PF_Z + T]
            r_sb = pf[:, PF_R : PF_R + T]
            xywh_sb = pf[:, PF_XYWH : PF_XYWH + 4 * T].rearrange(
                "p (t c) -> p t c", c=4
            )
            g_sb = pf[:, PF_G : PF_G + 4 * T].rearrange("p (t c) -> p t c", c=4)
            iota = pf[:, PF_IOTA : PF_IOTA + C]

            w_sb = constp.tile([128, T], f32)
            nc.vector.tensor_add(w_sb[:], z_sb, r_sb)
            rowmax = constp.tile([128, T], f32)
            lm = constp.tile([128, T], f32)
            out_sb = constp.tile([128, 2], f32)
            scr = constp.tile([128, T], f32)
            scr2 = constp.tile([128, T], f32)
            diff = constp.tile([128, T, 4], f32)
            dsum = constp.tile([128, T], f32)

            def build_nv(t, on_dve):
                """bf16 {0,1} mask nv[p,c] = (c != label[p,t])."""
                nv = nvp.tile([128, C], bf16)
                if on_dve:
                    nc.vector.tensor_scalar(
                        out=nv[:], in0=iota, scalar1=lab[:, t : t + 1],
                        scalar2=None, op0=OP.not_equal,
                    )
                else:
                    u = nvp.tile([128, C], f32)
                    nc.scalar.activation(
                        out=u[:], in_=iota, func=AF.Square,
                        scale=1.0, bias=nlab[:, t : t + 1],
                    )
                    nc.scalar.activation(out=nv[:], in_=u[:], func=AF.Sign)
                return nv

            for rep in range(reps):
                for g in range(NG):
                    t0 = g * G
                    # masks for the Pool tiles of this group (only need pf)
                    nvs = {}
                    for h in range(DVE_APPLY, G):
                        dve_nv = (h == DVE_APPLY) and (g % 2 == 0)
                        nvs[h] = build_nv(t0 + h, dve_nv)

                    csw = csp.tile([128, G, C], f32)
                    eng = nc.sync if g % 2 == 0 else nc.scalar
                    eng.dma_start(
                        out=csw[:],
                        in_=cs_d[:, t0 * C : (t0 + G) * C].rearrange(
                            "p (a c) -> p a c", c=C
                        ),
                    )
                    for h in range(DVE_APPLY):
                        t = t0 + h
                        msk = mskp.tile([128, C], f32)
                        nc.vector.scalar_tensor_tensor(
                            out=msk[:], in0=iota, scalar=lab[:, t : t + 1],
                            in1=csw[:, h, :], op0=OP.not_equal, op1=OP.mult,
                        )
                        nc.vector.reduce_max(
                            rowmax[:, t : t + 1], msk[:], axis=AX.X
                        )
                    for h in range(DVE_APPLY, G):
                        nc.gpsimd.tensor_tensor(
                            out=csw[:, h, :], in0=csw[:, h, :],
                            in1=nvs[h][:], op=OP.mult,
                        )
                    nc.vector.reduce_max(
                        rowmax[:, t0 + DVE_APPLY : t0 + G],
                        csw[:, DVE_APPLY:G, :], axis=AX.X,
                    )

                # epilogue: partial sums
                nc.scalar.activation(out=lm[:], in_=rowmax[:], func=AF.Ln)
                nc.vector.scalar_tensor_tensor(
                    out=scr[:], in0=w_sb[:], scalar=0.0, in1=lm[:],
                    op0=OP.bypass, op1=OP.mult, accum_out=out_sb[:, 0:1],
                )
                nc.vector.tensor_sub(diff[:], xywh_sb, g_sb)
                nc.vector.tensor_mul(diff[:], diff[:], diff[:])
                nc.vector.reduce_sum(dsum[:], diff[:], axis=AX.X)
                nc.vector.scalar_tensor_tensor(
                    out=scr2[:], in0=z_sb, scalar=0.0, in1=dsum[:],
                    op0=OP.bypass, op1=OP.mult, accum_out=out_sb[:, 1:2],
                )
            nc.sync.dma_start(out=out_d[:], in_=out_sb[:])

    nc.compile()
    return nc


def make_in_maps(class_scores, xywh, z, r, nearest_gt_idx, gt_class_labels, gt_xywh):
    cs = np.ascontiguousarray(np.asarray(class_scores, dtype=np.float32))
    xywh = np.ascontiguousarray(np.asarray(xywh, dtype=np.float32))
    z = np.ascontiguousarray(np.asarray(z, dtype=np.float32))
    r = np.ascontiguousarray(np.asarray(r, dtype=np.float32))
    idx = np.asarray(nearest_gt_idx).astype(np.int64)
    labels = np.asarray(gt_class_labels).astype(np.float32)[idx]       # [N]
    gx = np.asarray(gt_xywh, dtype=np.float32)[idx]                    # [N,4]

    iota_row = np.arange(C, dtype=np.float32)[None, :]
    in_maps = []
    for c in range(NCORES):
        lo, hi = c * R, (c + 1) * R
        if hi <= N:
            cs_s = cs[lo:hi]
            lab_s, z_s, r_s = labels[lo:hi], z[lo:hi], r[lo:hi]
            xywh_s, gx_s = xywh[lo:hi], gx[lo:hi]
        else:
            n_real = N - lo
            cs_s = np.ones((R, C), dtype=np.float32)
            cs_s[:n_real] = cs[lo:]
            lab_s = np.zeros(R, np.float32); lab_s[:n_real] = labels[lo:]
            z_s = np.zeros(R, np.float32); z_s[:n_real] = z[lo:]
            r_s = np.zeros(R, np.float32); r_s[:n_real] = r[lo:]
            xywh_s = np.zeros((R, 4), np.float32); xywh_s[:n_real] = xywh[lo:]
            gx_s = np.zeros((R, 4), np.float32); gx_s[:n_real] = gx[lo:]
        pf = np.empty((128, PF_COLS), dtype=np.float32)
        pf[:, PF_LAB : PF_LAB + T] = lab_s.reshape(128, T)
        pf[:, PF_NLAB : PF_NLAB + T] = -lab_s.reshape(128, T)
        pf[:, PF_Z : PF_Z + T] = z_s.reshape(128, T)
        pf[:, PF_R : PF_R + T] = r_s.reshape(128, T)
        pf[:, PF_XYWH : PF_XYWH + 4 * T] = xywh_s.reshape(128, 4 * T)
        pf[:, PF_G : PF_G + 4 * T] = gx_s.reshape(128, 4 * T)
        pf[:, PF_IOTA : PF_IOTA + C] = iota_row
        in_maps.append({"cs": cs_s.reshape(128, T * C), "pf": pf})
    return in_maps


def combine_outputs(outs):
    """outs: list of [128, 2] per-core partials -> final [1] float32."""
    partA = float(sum(o[:, 0].astype(np.float64).sum() for o in outs))
    partB = float(sum(o[:, 1].astype(np.float64).sum() for o in outs))
    with np.errstate(over="ignore", under="ignore"):
        tps = np.exp(-partB)
    val = -partA + tps
    return np.array([val], dtype=np.float32)


_NC_CACHE = None


def get_nc():
    global _NC_CACHE
    if _NC_CACHE is None:
        _NC_CACHE = build_nc()
    return _NC_CACHE


def kernel(**inputs) -> np.ndarray:
    nc = get_nc()
    in_maps = make_in_maps(**inputs)
    res = run_bass_kernel_spmd(nc, in_maps, core_ids=list(range(NCORES)))
    return combine_outputs([res.results[c]["out"] for c in range(NCORES)])
